# revision 7
# baseline (speedup 1.0000x reference)
"""Trainium2 Bass kernel for soft attention (show-attend-tell style).

reference math (per batch b):
    att1 = enc[b] @ W_enc + b_enc          # [P, A]
    att2 = dec[b] @ W_dec + b_dec          # [A]
    s    = relu(att1 + att2)               # [P, A]
    att  = s @ W_full[:, 0] (+ b_full)     # [P]   (b_full cancels in softmax)
    alpha = softmax(att)                   # [P]
    awe  = alpha @ enc[b]                  # [E]
returns (awe [B, E], alpha [B, P]) both fp32.

Sharding: pure data parallel over batch, 8 cores x 32 batches.
"""

from contextlib import ExitStack

import numpy as np

import concourse.bass as bass
import concourse.mybir as mybir
import concourse.tile as tile
from concourse import bacc
from concourse.masks import make_identity

F32 = mybir.dt.float32
BF16 = mybir.dt.bfloat16

B, P, E, D, A = 256, 196, 2048, 512, 512
N_CORES = 8
P_PAD = 208  # 196 padded to multiple of 16 for dma transpose
EK = E // 128  # 16 e-chunks
AK = A // 128  # 4 a-chunks
DK = D // 128  # 4 d-chunks


def build_nc(b_loc=B // N_CORES, group=4, debug=False):
    """Build the single-core Bass program (SPMD: every core runs this on its shard)."""
    nc = bacc.Bacc("TRN2", target_bir_lowering=False, debug=debug)

    enc = nc.declare_dram_parameter("encoder_out", [b_loc, P, E], F32, isOutput=False)
    dec = nc.declare_dram_parameter("decoder_hidden", [b_loc, D], F32, isOutput=False)
    w_enc = nc.declare_dram_parameter("W_enc", [E, A], F32, isOutput=False)
    b_enc = nc.declare_dram_parameter("b_enc", [A], F32, isOutput=False)
    w_dec = nc.declare_dram_parameter("W_dec", [D, A], F32, isOutput=False)
    b_dec = nc.declare_dram_parameter("b_dec", [D], F32, isOutput=False)
    w_full = nc.declare_dram_parameter("W_full", [A, 1], F32, isOutput=False)
    awe_out = nc.declare_dram_parameter("awe", [b_loc, E], F32, isOutput=True)
    alpha_out = nc.declare_dram_parameter("alpha", [b_loc, P], F32, isOutput=True)

    n_groups = b_loc // group
    assert n_groups * group == b_loc

    with tile.TileContext(nc) as tc, ExitStack() as ctx:
        consts = ctx.enter_context(tc.tile_pool(name="consts", bufs=1))
        enc_pool = ctx.enter_context(tc.tile_pool(name="enc", bufs=2 * group))
        encT_pool = ctx.enter_context(tc.tile_pool(name="encT", bufs=3))
        s_pool = ctx.enter_context(tc.tile_pool(name="s", bufs=3))
        sm_pool = ctx.enter_context(tc.tile_pool(name="sm", bufs=2))
        alphaT_pool = ctx.enter_context(tc.tile_pool(name="alphaT", bufs=2))
        awe_sb_pool = ctx.enter_context(tc.tile_pool(name="awe_sb", bufs=2))

        att1_ps = ctx.enter_context(tc.tile_pool(name="att1_ps", bufs=3, space="PSUM"))
        attT_ps = ctx.enter_context(tc.tile_pool(name="attT_ps", bufs=1, space="PSUM"))
        attT_ps2 = ctx.enter_context(tc.tile_pool(name="attT_ps2", bufs=1, space="PSUM"))
        small_ps = ctx.enter_context(tc.tile_pool(name="small_ps", bufs=2, space="PSUM"))
        awe_ps = ctx.enter_context(tc.tile_pool(name="awe_ps", bufs=1, space="PSUM"))

        # ---- constants / preprocessing ----
        identity = consts.tile([128, 128], F32, tag="identity")
        make_identity(nc, identity[:])

        # W_enc -> bf16, e-chunked: [128, EK, A]
        w_enc_bf = consts.tile([128, EK, A], BF16, tag="w_enc_bf")
        nc.gpsimd.dma_start(w_enc_bf[:], w_enc.rearrange("(k p) a -> p k a", p=128))
        # W_dec -> bf16 d-chunked
        w_dec_bf = consts.tile([128, DK, A], BF16, tag="w_dec_bf")
        nc.gpsimd.dma_start(w_dec_bf[:], w_dec.rearrange("(k p) a -> p k a", p=128))
        # W_full -> bf16 a-chunked column [128, AK]
        w_full_bf = consts.tile([128, AK], BF16, tag="w_full_bf")
        nc.gpsimd.dma_start(w_full_bf[:], w_full.rearrange("(k p) o -> p (k o)", p=128))
        # bias = b_enc + b_dec as per-partition columns [128, AK]
        b_enc_sb = consts.tile([128, AK], F32, tag="b_enc_sb")
        nc.sync.dma_start(b_enc_sb[:], b_enc.rearrange("(k p) -> p k", p=128))
        b_dec_sb = consts.tile([128, AK], F32, tag="b_dec_sb")
        nc.sync.dma_start(b_dec_sb[:], b_dec.rearrange("(k p) -> p k", p=128))
        bias_pp = consts.tile([128, AK], F32, tag="bias_pp")
        nc.vector.tensor_add(bias_pp[:], b_enc_sb[:], b_dec_sb[:])

        # decoder hidden: [b_loc, D] -> transpose -> bf16 [128, DK, b_loc]
        dec_sb = consts.tile([b_loc, D], F32, tag="dec_sb")
        nc.sync.dma_start(dec_sb[:], dec[:, :])
        decT_bf = consts.tile([128, DK, b_loc], BF16, tag="decT_bf")
        for k in range(DK):
            ps = small_ps.tile([128, b_loc], F32, tag="small")
            nc.tensor.transpose(
                ps[:], dec_sb[:, k * 128 : (k + 1) * 128], identity[:b_loc, :b_loc]
            )
            nc.scalar.copy(decT_bf[:, k, :], ps[:])

        # att2' = dec @ W_dec + (b_dec + b_enc), transposed: [128, AK, b_loc] fp32
        att2pp = consts.tile([128, AK, b_loc], F32, tag="att2pp")
        for m in range(AK):
            ps = small_ps.tile([128, b_loc], F32, tag="small")
            for k in range(DK):
                nc.tensor.matmul(
                    ps[:],
                    lhsT=w_dec_bf[:, k, m * 128 : (m + 1) * 128],
                    rhs=decT_bf[:, k, :],
                    start=(k == 0),
                    stop=(k == DK - 1),
                )
            nc.scalar.activation(
                att2pp[:, m, :],
                ps[:],
                mybir.ActivationFunctionType.Identity,
                bias=bias_pp[:, m : m + 1],
                scale=1.0,
            )

        # ---- main loop over groups of batches, processed in pairs ----
        assert group % 2 == 0
        for g in range(n_groups):
            attT0 = attT_ps.tile([128, group], F32, tag="attT0")
            attT1 = attT_ps2.tile([68, group], F32, tag="attT1")
            enc_tiles = []
            s_tiles = []
            for pi in range(group // 2):
                # two batches share one encT tile + one att1 psum accumulation
                encT = encT_pool.tile([128, EK, 2 * P_PAD], BF16, tag="encT")
                for h in range(2):
                    b = g * group + pi * 2 + h
                    # load + cast fp32 -> bf16 (SWDGE), natural layout [p, e]
                    enc_bf = enc_pool.tile([128, 2, E], BF16, tag="enc_bf")
                    # zero rows 64:80 of block 1 first (engine partition starts must
                    # be 32-aligned); the load below overwrites 64:68 with real data,
                    # leaving the 68:80 dma-transpose pad rows defined.
                    nc.vector.memset(enc_bf[64:80, 1, :], 0.0)
                    nc.gpsimd.dma_start(enc_bf[:, 0, :], enc[b, 0:128, :])
                    nc.gpsimd.dma_start(enc_bf[:68, 1, :], enc[b, 128:P, :])
                    enc_tiles.append(enc_bf)
                    # transpose to [e, p]: encT[q, k, h*P_PAD+p] = enc[p, 128k+q]
                    off = h * P_PAD
                    nc.sync.dma_start_transpose(
                        encT[:, :, off : off + 128], enc_bf[:, 0, :]
                    )
                    nc.sync.dma_start_transpose(
                        encT[:, :, off + 128 : off + P_PAD], enc_bf[:80, 1, :]
                    )

                # att1^T for the pair, accumulated over e-chunks; fused relu+bias
                encT_pair = encT[:].rearrange("q k (h p) -> q k h p", h=2)
                s01 = [
                    s_pool.tile([128, AK, P], BF16, tag="s_bf", name=f"s_{g}_{pi}_{h}")
                    for h in range(2)
                ]
                s_tiles.extend(s01)
                for m in range(AK):
                    ps1 = att1_ps.tile([128, 2, P], F32, tag="att1")
                    for k in range(EK):
                        nc.tensor.matmul(
                            ps1[:],
                            lhsT=w_enc_bf[:, k, m * 128 : (m + 1) * 128],
                            rhs=encT_pair[:, k, :, 0:P],
                            start=(k == 0),
                            stop=(k == EK - 1),
                        )
                    for h in range(2):
                        b = g * group + pi * 2 + h
                        nc.scalar.activation(
                            s01[h][:, m, :],
                            ps1[:, h, :],
                            mybir.ActivationFunctionType.Relu,
                            bias=att2pp[:, m, b : b + 1],
                            scale=1.0,
                        )

            # att^T columns: att[p] = sum_a s[a, p] * w_full[a]
            for bi in range(group):
                s_bf = s_tiles[bi]
                for m in range(AK):
                    nc.tensor.matmul(
                        attT0[:, bi : bi + 1],
                        lhsT=s_bf[:, m, 0:128],
                        rhs=w_full_bf[:, m : m + 1],
                        start=(m == 0),
                        stop=(m == AK - 1),
                    )
                for m in range(AK):
                    nc.tensor.matmul(
                        attT1[:, bi : bi + 1],
                        lhsT=s_bf[:, m, 128:P],
                        rhs=w_full_bf[:, m : m + 1],
                        start=(m == 0),
                        stop=(m == AK - 1),
                    )

            # ---- group softmax (no max-subtraction: |att| <~ 2, exp is safe) ----
            expT0 = sm_pool.tile([128, group], F32, tag="expT0")
            nc.scalar.activation(expT0[:], attT0[:], mybir.ActivationFunctionType.Exp)
            expT1 = sm_pool.tile([68, group], F32, tag="expT1")
            nc.scalar.activation(expT1[:], attT1[:], mybir.ActivationFunctionType.Exp)

            psR0 = small_ps.tile([group, 128], F32, tag="small")
            nc.tensor.transpose(psR0[:], expT0[:], identity[:128, :128])
            psR1 = small_ps.tile([group, 68], F32, tag="small")
            nc.tensor.transpose(psR1[:], expT1[:], identity[:68, :68])

            exp_rows = sm_pool.tile([group, P], F32, tag="exp_rows")
            nc.scalar.copy(exp_rows[:, 0:128], psR0[:])
            nc.scalar.copy(exp_rows[:, 128:P], psR1[:])

            sums = sm_pool.tile([group, 1], F32, tag="sums")
            nc.vector.tensor_reduce(
                sums[:], exp_rows[:], axis=mybir.AxisListType.X, op=mybir.AluOpType.add
            )
            rcp = sm_pool.tile([group, 1], F32, tag="rcp")
            nc.vector.reciprocal(rcp[:], sums[:])
            alpha_rows = sm_pool.tile([group, P], F32, tag="alpha_rows")
            nc.vector.tensor_scalar_mul(alpha_rows[:], exp_rows[:], rcp[:, 0:1])
            nc.sync.dma_start(alpha_out[g * group : (g + 1) * group, :], alpha_rows[:])

            # alpha^T in bf16 (stationary operand for awe matmuls)
            psT0 = small_ps.tile([128, group], F32, tag="small")
            nc.tensor.transpose(psT0[:], alpha_rows[:, 0:128], identity[:group, :group])
            alphaT0 = alphaT_pool.tile([128, group], BF16, tag="alphaT0")
            nc.scalar.copy(alphaT0[:], psT0[:])
            psT1 = small_ps.tile([68, group], F32, tag="small")
            nc.tensor.transpose(psT1[:], alpha_rows[:, 128:P], identity[:group, :group])
            alphaT1 = alphaT_pool.tile([68, group], BF16, tag="alphaT1")
            nc.scalar.copy(alphaT1[:], psT1[:])

            # ---- awe: awe[b, e] = sum_p alpha[b, p] * enc[b, p, e] ----
            awe_strip = awe_sb_pool.tile([1, group * E], F32, tag="awe_strip")
            for bi in range(group):
                for sl in range(E // 512):
                    psA = awe_ps.tile([1, 512], F32, tag="awe")
                    nc.tensor.matmul(
                        psA[:],
                        lhsT=alphaT0[:, bi : bi + 1],
                        rhs=enc_tiles[bi][:, 0, sl * 512 : (sl + 1) * 512],
                        start=True,
                        stop=False,
                    )
                    nc.tensor.matmul(
                        psA[:],
                        lhsT=alphaT1[:, bi : bi + 1],
                        rhs=enc_tiles[bi][:68, 1, sl * 512 : (sl + 1) * 512],
                        start=False,
                        stop=True,
                    )
                    evict = nc.scalar.copy if sl % 2 == 0 else nc.vector.tensor_copy
                    evict(
                        awe_strip[0:1, bi * E + sl * 512 : bi * E + (sl + 1) * 512],
                        psA[:],
                    )
            nc.sync.dma_start(
                awe_out[g * group : (g + 1) * group, :], awe_strip[0:1, :]
            )

    nc.compile()
    return nc


_NC_CACHE = {}


def _get_nc(b_loc, group):
    key = (b_loc, group)
    if key not in _NC_CACHE:
        _NC_CACHE[key] = build_nc(b_loc, group)
    return _NC_CACHE[key]


def run_spmd(inputs, trace=False, **kwargs):
    """Run on 8 NeuronCores; returns (awe, alpha, BassKernelResults)."""
    from concourse import bass_utils

    enc = np.asarray(inputs["encoder_out"], dtype=np.float32)
    dec = np.asarray(inputs["decoder_hidden"], dtype=np.float32)
    shared = {
        "W_enc": np.asarray(inputs["W_enc"], dtype=np.float32),
        "b_enc": np.asarray(inputs["b_enc"], dtype=np.float32),
        "W_dec": np.asarray(inputs["W_dec"], dtype=np.float32),
        "b_dec": np.asarray(inputs["b_dec"], dtype=np.float32),
        "W_full": np.asarray(inputs["W_full"], dtype=np.float32),
    }
    b_total = enc.shape[0]
    b_loc = b_total // N_CORES
    nc = _get_nc(b_loc, 4)

    in_maps = []
    for c in range(N_CORES):
        sl = slice(c * b_loc, (c + 1) * b_loc)
        m = dict(shared)
        m["encoder_out"] = np.ascontiguousarray(enc[sl])
        m["decoder_hidden"] = np.ascontiguousarray(dec[sl])
        in_maps.append(m)

    res = bass_utils.run_bass_kernel_spmd(
        nc, in_maps, list(range(N_CORES)), trace=trace, **kwargs
    )
    awe = np.concatenate([r["awe"] for r in res.results], axis=0)
    alpha = np.concatenate([r["alpha"] for r in res.results], axis=0)
    return awe, alpha, res


def kernel(**inputs):
    awe, alpha, _ = run_spmd(inputs)
    return awe, alpha


# revision 9
# speedup vs baseline: 265.8185x; 265.8185x over previous
"""Trainium2 Bass kernel for soft attention (show-attend-tell style).

reference math (per batch b):
    att1 = enc[b] @ W_enc + b_enc          # [P, A]
    att2 = dec[b] @ W_dec + b_dec          # [A]
    s    = relu(att1 + att2)               # [P, A]
    att  = s @ W_full[:, 0] (+ b_full)     # [P]   (b_full cancels in softmax)
    alpha = softmax(att)                   # [P]
    awe  = alpha @ enc[b]                  # [E]
returns (awe [B, E], alpha [B, P]) both fp32.

Sharding: pure data parallel over batch, 8 cores x 32 batches.
"""

from contextlib import ExitStack

import numpy as np

import concourse.bass as bass
import concourse.mybir as mybir
import concourse.tile as tile
from concourse import bacc
from concourse.masks import make_identity

F32 = mybir.dt.float32
BF16 = mybir.dt.bfloat16

B, P, E, D, A = 256, 196, 2048, 512, 512
N_CORES = 8
P_PAD = 208  # 196 padded to multiple of 16 for dma transpose
EK = E // 128  # 16 e-chunks
AK = A // 128  # 4 a-chunks
DK = D // 128  # 4 d-chunks


def build_nc(b_loc=B // N_CORES, group=4, debug=False, repeats=1):
    """Build the single-core Bass program (SPMD: every core runs this on its shard)."""
    nc = bacc.Bacc("TRN2", target_bir_lowering=False, debug=debug)

    enc = nc.declare_dram_parameter("encoder_out", [b_loc, P, E], F32, isOutput=False)
    dec = nc.declare_dram_parameter("decoder_hidden", [b_loc, D], F32, isOutput=False)
    w_enc = nc.declare_dram_parameter("W_enc", [E, A], F32, isOutput=False)
    b_enc = nc.declare_dram_parameter("b_enc", [A], F32, isOutput=False)
    w_dec = nc.declare_dram_parameter("W_dec", [D, A], F32, isOutput=False)
    b_dec = nc.declare_dram_parameter("b_dec", [D], F32, isOutput=False)
    w_full = nc.declare_dram_parameter("W_full", [A, 1], F32, isOutput=False)
    awe_out = nc.declare_dram_parameter("awe", [b_loc, E], F32, isOutput=True)
    alpha_out = nc.declare_dram_parameter("alpha", [b_loc, P], F32, isOutput=True)

    n_groups = b_loc // group
    assert n_groups * group == b_loc

    with tile.TileContext(nc) as tc, ExitStack() as ctx:
        consts = ctx.enter_context(tc.tile_pool(name="consts", bufs=1))
        enc_pool = ctx.enter_context(tc.tile_pool(name="enc", bufs=2 * group))
        encT_pool = ctx.enter_context(tc.tile_pool(name="encT", bufs=3))
        s_pool = ctx.enter_context(tc.tile_pool(name="s", bufs=3))
        sm_pool = ctx.enter_context(tc.tile_pool(name="sm", bufs=2))
        alphaT_pool = ctx.enter_context(tc.tile_pool(name="alphaT", bufs=2))
        awe_sb_pool = ctx.enter_context(tc.tile_pool(name="awe_sb", bufs=2))

        att1_ps = ctx.enter_context(tc.tile_pool(name="att1_ps", bufs=3, space="PSUM"))
        attT_ps = ctx.enter_context(tc.tile_pool(name="attT_ps", bufs=1, space="PSUM"))
        attT_ps2 = ctx.enter_context(tc.tile_pool(name="attT_ps2", bufs=1, space="PSUM"))
        small_ps = ctx.enter_context(tc.tile_pool(name="small_ps", bufs=2, space="PSUM"))
        awe_ps = ctx.enter_context(tc.tile_pool(name="awe_ps", bufs=1, space="PSUM"))

        # ---- constants / preprocessing ----
        identity = consts.tile([128, 128], F32, tag="identity")
        make_identity(nc, identity[:])

        # W_enc -> bf16, e-chunked: [128, EK, A]
        w_enc_bf = consts.tile([128, EK, A], BF16, tag="w_enc_bf")
        nc.gpsimd.dma_start(w_enc_bf[:], w_enc.rearrange("(k p) a -> p k a", p=128))
        # W_dec -> bf16 d-chunked
        w_dec_bf = consts.tile([128, DK, A], BF16, tag="w_dec_bf")
        nc.gpsimd.dma_start(w_dec_bf[:], w_dec.rearrange("(k p) a -> p k a", p=128))
        # W_full -> bf16 a-chunked column [128, AK]
        w_full_bf = consts.tile([128, AK], BF16, tag="w_full_bf")
        nc.gpsimd.dma_start(w_full_bf[:], w_full.rearrange("(k p) o -> p (k o)", p=128))
        # bias = b_enc + b_dec as per-partition columns [128, AK]
        b_enc_sb = consts.tile([128, AK], F32, tag="b_enc_sb")
        nc.sync.dma_start(b_enc_sb[:], b_enc.rearrange("(k p) -> p k", p=128))
        b_dec_sb = consts.tile([128, AK], F32, tag="b_dec_sb")
        nc.sync.dma_start(b_dec_sb[:], b_dec.rearrange("(k p) -> p k", p=128))
        bias_pp = consts.tile([128, AK], F32, tag="bias_pp")
        nc.vector.tensor_add(bias_pp[:], b_enc_sb[:], b_dec_sb[:])

        # decoder hidden: [b_loc, D] -> transpose -> bf16 [128, DK, b_loc]
        dec_sb = consts.tile([b_loc, D], F32, tag="dec_sb")
        nc.sync.dma_start(dec_sb[:], dec[:, :])
        decT_bf = consts.tile([128, DK, b_loc], BF16, tag="decT_bf")
        for k in range(DK):
            ps = small_ps.tile([128, b_loc], F32, tag="small")
            nc.tensor.transpose(
                ps[:], dec_sb[:, k * 128 : (k + 1) * 128], identity[:b_loc, :b_loc]
            )
            nc.scalar.copy(decT_bf[:, k, :], ps[:])

        # att2' = dec @ W_dec + (b_dec + b_enc), transposed: [128, AK, b_loc] fp32
        att2pp = consts.tile([128, AK, b_loc], F32, tag="att2pp")
        for m in range(AK):
            ps = small_ps.tile([128, b_loc], F32, tag="small")
            for k in range(DK):
                nc.tensor.matmul(
                    ps[:],
                    lhsT=w_dec_bf[:, k, m * 128 : (m + 1) * 128],
                    rhs=decT_bf[:, k, :],
                    start=(k == 0),
                    stop=(k == DK - 1),
                )
            nc.scalar.activation(
                att2pp[:, m, :],
                ps[:],
                mybir.ActivationFunctionType.Identity,
                bias=bias_pp[:, m : m + 1],
                scale=1.0,
            )

        # ---- main loop over groups of batches, processed in pairs ----
        # (repeats > 1 re-runs the whole pipeline for benchmarking deltas)
        assert group % 2 == 0
        for g in range(n_groups * repeats):
            g = g % n_groups
            attT0 = attT_ps.tile([128, group], F32, tag="attT0")
            attT1 = attT_ps2.tile([68, group], F32, tag="attT1")
            enc_tiles = []
            s_tiles = []
            for pi in range(group // 2):
                # two batches share one encT tile + one att1 psum accumulation
                encT = encT_pool.tile([128, EK, 2 * P_PAD], BF16, tag="encT")
                for h in range(2):
                    b = g * group + pi * 2 + h
                    # load + cast fp32 -> bf16 (SWDGE), natural layout [p, e]
                    enc_bf = enc_pool.tile([128, 2, E], BF16, tag="enc_bf")
                    # zero rows 64:80 of block 1 first (engine partition starts must
                    # be 32-aligned); the load below overwrites 64:68 with real data,
                    # leaving the 68:80 dma-transpose pad rows defined.
                    nc.vector.memset(enc_bf[64:80, 1, :], 0.0)
                    nc.gpsimd.dma_start(enc_bf[:, 0, :], enc[b, 0:128, :])
                    nc.gpsimd.dma_start(enc_bf[:68, 1, :], enc[b, 128:P, :])
                    enc_tiles.append(enc_bf)
                    # transpose to [e, p]: encT[q, k, h*P_PAD+p] = enc[p, 128k+q]
                    off = h * P_PAD
                    nc.sync.dma_start_transpose(
                        encT[:, :, off : off + 128], enc_bf[:, 0, :]
                    )
                    nc.sync.dma_start_transpose(
                        encT[:, :, off + 128 : off + P_PAD], enc_bf[:80, 1, :]
                    )

                # att1^T for the pair, accumulated over e-chunks; fused relu+bias
                encT_pair = encT[:].rearrange("q k (h p) -> q k h p", h=2)
                s01 = [
                    s_pool.tile([128, AK, P], BF16, tag="s_bf", name=f"s_{g}_{pi}_{h}")
                    for h in range(2)
                ]
                s_tiles.extend(s01)
                for m in range(AK):
                    ps1 = att1_ps.tile([128, 2, P], F32, tag="att1")
                    for k in range(EK):
                        nc.tensor.matmul(
                            ps1[:],
                            lhsT=w_enc_bf[:, k, m * 128 : (m + 1) * 128],
                            rhs=encT_pair[:, k, :, 0:P],
                            start=(k == 0),
                            stop=(k == EK - 1),
                        )
                    for h in range(2):
                        b = g * group + pi * 2 + h
                        nc.scalar.activation(
                            s01[h][:, m, :],
                            ps1[:, h, :],
                            mybir.ActivationFunctionType.Relu,
                            bias=att2pp[:, m, b : b + 1],
                            scale=1.0,
                        )

            # att^T columns: att[p] = sum_a s[a, p] * w_full[a]
            for bi in range(group):
                s_bf = s_tiles[bi]
                for m in range(AK):
                    nc.tensor.matmul(
                        attT0[:, bi : bi + 1],
                        lhsT=s_bf[:, m, 0:128],
                        rhs=w_full_bf[:, m : m + 1],
                        start=(m == 0),
                        stop=(m == AK - 1),
                    )
                for m in range(AK):
                    nc.tensor.matmul(
                        attT1[:, bi : bi + 1],
                        lhsT=s_bf[:, m, 128:P],
                        rhs=w_full_bf[:, m : m + 1],
                        start=(m == 0),
                        stop=(m == AK - 1),
                    )

            # ---- group softmax (no max-subtraction: |att| <~ 2, exp is safe) ----
            expT0 = sm_pool.tile([128, group], F32, tag="expT0")
            nc.scalar.activation(expT0[:], attT0[:], mybir.ActivationFunctionType.Exp)
            expT1 = sm_pool.tile([68, group], F32, tag="expT1")
            nc.scalar.activation(expT1[:], attT1[:], mybir.ActivationFunctionType.Exp)

            psR0 = small_ps.tile([group, 128], F32, tag="small")
            nc.tensor.transpose(psR0[:], expT0[:], identity[:128, :128])
            psR1 = small_ps.tile([group, 68], F32, tag="small")
            nc.tensor.transpose(psR1[:], expT1[:], identity[:68, :68])

            exp_rows = sm_pool.tile([group, P], F32, tag="exp_rows")
            nc.scalar.copy(exp_rows[:, 0:128], psR0[:])
            nc.scalar.copy(exp_rows[:, 128:P], psR1[:])

            sums = sm_pool.tile([group, 1], F32, tag="sums")
            nc.vector.tensor_reduce(
                sums[:], exp_rows[:], axis=mybir.AxisListType.X, op=mybir.AluOpType.add
            )
            rcp = sm_pool.tile([group, 1], F32, tag="rcp")
            nc.vector.reciprocal(rcp[:], sums[:])
            alpha_rows = sm_pool.tile([group, P], F32, tag="alpha_rows")
            nc.vector.tensor_scalar_mul(alpha_rows[:], exp_rows[:], rcp[:, 0:1])
            nc.sync.dma_start(alpha_out[g * group : (g + 1) * group, :], alpha_rows[:])

            # alpha^T in bf16 (stationary operand for awe matmuls)
            psT0 = small_ps.tile([128, group], F32, tag="small")
            nc.tensor.transpose(psT0[:], alpha_rows[:, 0:128], identity[:group, :group])
            alphaT0 = alphaT_pool.tile([128, group], BF16, tag="alphaT0")
            nc.scalar.copy(alphaT0[:], psT0[:])
            psT1 = small_ps.tile([68, group], F32, tag="small")
            nc.tensor.transpose(psT1[:], alpha_rows[:, 128:P], identity[:group, :group])
            alphaT1 = alphaT_pool.tile([68, group], BF16, tag="alphaT1")
            nc.scalar.copy(alphaT1[:], psT1[:])

            # ---- awe: awe[b, e] = sum_p alpha[b, p] * enc[b, p, e] ----
            awe_strip = awe_sb_pool.tile([1, group * E], F32, tag="awe_strip")
            for bi in range(group):
                for sl in range(E // 512):
                    psA = awe_ps.tile([1, 512], F32, tag="awe")
                    nc.tensor.matmul(
                        psA[:],
                        lhsT=alphaT0[:, bi : bi + 1],
                        rhs=enc_tiles[bi][:, 0, sl * 512 : (sl + 1) * 512],
                        start=True,
                        stop=False,
                    )
                    nc.tensor.matmul(
                        psA[:],
                        lhsT=alphaT1[:, bi : bi + 1],
                        rhs=enc_tiles[bi][:68, 1, sl * 512 : (sl + 1) * 512],
                        start=False,
                        stop=True,
                    )
                    evict = nc.scalar.copy if sl % 2 == 0 else nc.vector.tensor_copy
                    evict(
                        awe_strip[0:1, bi * E + sl * 512 : bi * E + (sl + 1) * 512],
                        psA[:],
                    )
            nc.sync.dma_start(
                awe_out[g * group : (g + 1) * group, :], awe_strip[0:1, :]
            )

    nc.compile()
    return nc


_NC_CACHE = {}


def _get_nc(b_loc, group):
    key = (b_loc, group)
    if key not in _NC_CACHE:
        _NC_CACHE[key] = build_nc(b_loc, group)
    return _NC_CACHE[key]


def run_spmd(inputs, trace=False, **kwargs):
    """Run on 8 NeuronCores; returns (awe, alpha, BassKernelResults)."""
    from concourse import bass_utils

    enc = np.asarray(inputs["encoder_out"], dtype=np.float32)
    dec = np.asarray(inputs["decoder_hidden"], dtype=np.float32)
    shared = {
        "W_enc": np.asarray(inputs["W_enc"], dtype=np.float32),
        "b_enc": np.asarray(inputs["b_enc"], dtype=np.float32),
        "W_dec": np.asarray(inputs["W_dec"], dtype=np.float32),
        "b_dec": np.asarray(inputs["b_dec"], dtype=np.float32),
        "W_full": np.asarray(inputs["W_full"], dtype=np.float32),
    }
    b_total = enc.shape[0]
    b_loc = b_total // N_CORES
    nc = _get_nc(b_loc, 4)

    in_maps = []
    for c in range(N_CORES):
        sl = slice(c * b_loc, (c + 1) * b_loc)
        m = dict(shared)
        m["encoder_out"] = np.ascontiguousarray(enc[sl])
        m["decoder_hidden"] = np.ascontiguousarray(dec[sl])
        in_maps.append(m)

    res = bass_utils.run_bass_kernel_spmd(
        nc, in_maps, list(range(N_CORES)), trace=trace, **kwargs
    )
    awe = np.concatenate([r["awe"] for r in res.results], axis=0)
    alpha = np.concatenate([r["alpha"] for r in res.results], axis=0)
    return awe, alpha, res


def kernel(**inputs):
    awe, alpha, _ = run_spmd(inputs)
    return awe, alpha


# revision 11
# speedup vs baseline: 408.0509x; 1.5351x over previous
"""Trainium2 Bass kernel for soft attention (show-attend-tell style).

reference math (per batch b):
    att1 = enc[b] @ W_enc + b_enc          # [P, A]
    att2 = dec[b] @ W_dec + b_dec          # [A]
    s    = relu(att1 + att2)               # [P, A]
    att  = s @ W_full[:, 0] (+ b_full)     # [P]   (b_full cancels in softmax)
    alpha = softmax(att)                   # [P]
    awe  = alpha @ enc[b]                  # [E]
returns (awe [B, E], alpha [B, P]) both fp32.

Sharding: pure data parallel over batch, 8 cores x 32 batches.
"""

from contextlib import ExitStack

import numpy as np

import concourse.bass as bass
import concourse.mybir as mybir
import concourse.tile as tile
from concourse import bacc
from concourse.masks import make_identity

F32 = mybir.dt.float32
BF16 = mybir.dt.bfloat16

B, P, E, D, A = 256, 196, 2048, 512, 512
N_CORES = 8
P_PAD = 208  # 196 padded to multiple of 16 for dma transpose
EK = E // 128  # 16 e-chunks
AK = A // 128  # 4 a-chunks
DK = D // 128  # 4 d-chunks


def build_nc(b_loc=B // N_CORES, group=4, debug=False, repeats=1, **opt):
    """Build the single-core Bass program (SPMD: every core runs this on its shard)."""
    o = dict(
        enc_bufs=2 * group,
        encT_bufs=3,
        s_bufs=3,
        att1_bufs=3,
        small_bufs=2,
        awe_bufs=1,
        attT_bufs=1,
    )
    o.update(opt)
    nc = bacc.Bacc("TRN2", target_bir_lowering=False, debug=debug)

    enc = nc.declare_dram_parameter("encoder_out", [b_loc, P, E], F32, isOutput=False)
    dec = nc.declare_dram_parameter("decoder_hidden", [b_loc, D], F32, isOutput=False)
    w_enc = nc.declare_dram_parameter("W_enc", [E, A], F32, isOutput=False)
    b_enc = nc.declare_dram_parameter("b_enc", [A], F32, isOutput=False)
    w_dec = nc.declare_dram_parameter("W_dec", [D, A], F32, isOutput=False)
    b_dec = nc.declare_dram_parameter("b_dec", [D], F32, isOutput=False)
    w_full = nc.declare_dram_parameter("W_full", [A, 1], F32, isOutput=False)
    awe_out = nc.declare_dram_parameter("awe", [b_loc, E], F32, isOutput=True)
    alpha_out = nc.declare_dram_parameter("alpha", [b_loc, P], F32, isOutput=True)

    n_groups = b_loc // group
    assert n_groups * group == b_loc

    with tile.TileContext(nc) as tc, ExitStack() as ctx:
        consts = ctx.enter_context(tc.tile_pool(name="consts", bufs=1))
        enc_pool = ctx.enter_context(tc.tile_pool(name="enc", bufs=o["enc_bufs"]))
        encT_pool = ctx.enter_context(tc.tile_pool(name="encT", bufs=o["encT_bufs"]))
        s_pool = ctx.enter_context(tc.tile_pool(name="s", bufs=o["s_bufs"]))
        sm_pool = ctx.enter_context(tc.tile_pool(name="sm", bufs=2))
        alphaT_pool = ctx.enter_context(tc.tile_pool(name="alphaT", bufs=2))
        awe_sb_pool = ctx.enter_context(tc.tile_pool(name="awe_sb", bufs=2))

        att1_ps = ctx.enter_context(
            tc.tile_pool(name="att1_ps", bufs=o["att1_bufs"], space="PSUM")
        )
        attT_ps = ctx.enter_context(
            tc.tile_pool(name="attT_ps", bufs=o["attT_bufs"], space="PSUM")
        )
        attT_ps2 = ctx.enter_context(
            tc.tile_pool(name="attT_ps2", bufs=o["attT_bufs"], space="PSUM")
        )
        small_ps = ctx.enter_context(
            tc.tile_pool(name="small_ps", bufs=o["small_bufs"], space="PSUM")
        )
        awe_ps = ctx.enter_context(
            tc.tile_pool(name="awe_ps", bufs=o["awe_bufs"], space="PSUM")
        )

        # ---- constants / preprocessing ----
        identity = consts.tile([128, 128], F32, tag="identity")
        make_identity(nc, identity[:])

        # W_enc -> bf16, e-chunked: [128, EK, A]
        w_enc_bf = consts.tile([128, EK, A], BF16, tag="w_enc_bf")
        nc.gpsimd.dma_start(w_enc_bf[:], w_enc.rearrange("(k p) a -> p k a", p=128))
        # W_dec -> bf16 d-chunked
        w_dec_bf = consts.tile([128, DK, A], BF16, tag="w_dec_bf")
        nc.gpsimd.dma_start(w_dec_bf[:], w_dec.rearrange("(k p) a -> p k a", p=128))
        # W_full -> bf16 a-chunked column [128, AK]
        w_full_bf = consts.tile([128, AK], BF16, tag="w_full_bf")
        nc.gpsimd.dma_start(w_full_bf[:], w_full.rearrange("(k p) o -> p (k o)", p=128))
        # bias = b_enc + b_dec as per-partition columns [128, AK]
        b_enc_sb = consts.tile([128, AK], F32, tag="b_enc_sb")
        nc.sync.dma_start(b_enc_sb[:], b_enc.rearrange("(k p) -> p k", p=128))
        b_dec_sb = consts.tile([128, AK], F32, tag="b_dec_sb")
        nc.sync.dma_start(b_dec_sb[:], b_dec.rearrange("(k p) -> p k", p=128))
        bias_pp = consts.tile([128, AK], F32, tag="bias_pp")
        nc.vector.tensor_add(bias_pp[:], b_enc_sb[:], b_dec_sb[:])

        # decoder hidden: [b_loc, D] -> transpose -> bf16 [128, DK, b_loc]
        dec_sb = consts.tile([b_loc, D], F32, tag="dec_sb")
        nc.sync.dma_start(dec_sb[:], dec[:, :])
        decT_bf = consts.tile([128, DK, b_loc], BF16, tag="decT_bf")
        for k in range(DK):
            ps = small_ps.tile([128, b_loc], F32, tag="small")
            nc.tensor.transpose(
                ps[:], dec_sb[:, k * 128 : (k + 1) * 128], identity[:b_loc, :b_loc]
            )
            nc.scalar.copy(decT_bf[:, k, :], ps[:])

        # att2' = dec @ W_dec + (b_dec + b_enc), transposed: [128, AK, b_loc] fp32
        att2pp = consts.tile([128, AK, b_loc], F32, tag="att2pp")
        for m in range(AK):
            ps = small_ps.tile([128, b_loc], F32, tag="small")
            for k in range(DK):
                nc.tensor.matmul(
                    ps[:],
                    lhsT=w_dec_bf[:, k, m * 128 : (m + 1) * 128],
                    rhs=decT_bf[:, k, :],
                    start=(k == 0),
                    stop=(k == DK - 1),
                )
            nc.scalar.activation(
                att2pp[:, m, :],
                ps[:],
                mybir.ActivationFunctionType.Identity,
                bias=bias_pp[:, m : m + 1],
                scale=1.0,
            )

        # ---- main loop over groups of batches, processed in pairs ----
        # (repeats > 1 re-runs the whole pipeline for benchmarking deltas)
        assert group % 2 == 0
        for g in range(n_groups * repeats):
            g = g % n_groups
            attT0 = attT_ps.tile([128, group], F32, tag="attT0")
            attT1 = attT_ps2.tile([68, group], F32, tag="attT1")
            enc_tiles = []
            s_tiles = []
            for pi in range(group // 2):
                # two batches share one encT tile + one att1 psum accumulation
                encT = encT_pool.tile([128, EK, 2 * P_PAD], BF16, tag="encT")
                for h in range(2):
                    b = g * group + pi * 2 + h
                    # load + cast fp32 -> bf16 (SWDGE), natural layout [p, e]
                    enc_bf = enc_pool.tile([128, 2, E], BF16, tag="enc_bf")
                    # zero rows 64:80 of block 1 first (engine partition starts must
                    # be 32-aligned); the load below overwrites 64:68 with real data,
                    # leaving the 68:80 dma-transpose pad rows defined.
                    nc.vector.memset(enc_bf[64:80, 1, :], 0.0)
                    nc.gpsimd.dma_start(enc_bf[:, 0, :], enc[b, 0:128, :])
                    nc.gpsimd.dma_start(enc_bf[:68, 1, :], enc[b, 128:P, :])
                    enc_tiles.append(enc_bf)
                    # transpose to [e, p]: encT[q, k, h*P_PAD+p] = enc[p, 128k+q]
                    off = h * P_PAD
                    nc.sync.dma_start_transpose(
                        encT[:, :, off : off + 128], enc_bf[:, 0, :]
                    )
                    nc.sync.dma_start_transpose(
                        encT[:, :, off + 128 : off + P_PAD], enc_bf[:80, 1, :]
                    )

                # att1^T for the pair, accumulated over e-chunks; fused relu+bias
                encT_pair = encT[:].rearrange("q k (h p) -> q k h p", h=2)
                s01 = [
                    s_pool.tile([128, AK, P], BF16, tag="s_bf", name=f"s_{g}_{pi}_{h}")
                    for h in range(2)
                ]
                s_tiles.extend(s01)
                for m in range(AK):
                    ps1 = att1_ps.tile([128, 2, P], F32, tag="att1")
                    for k in range(EK):
                        nc.tensor.matmul(
                            ps1[:],
                            lhsT=w_enc_bf[:, k, m * 128 : (m + 1) * 128],
                            rhs=encT_pair[:, k, :, 0:P],
                            start=(k == 0),
                            stop=(k == EK - 1),
                        )
                    for h in range(2):
                        b = g * group + pi * 2 + h
                        nc.scalar.activation(
                            s01[h][:, m, :],
                            ps1[:, h, :],
                            mybir.ActivationFunctionType.Relu,
                            bias=att2pp[:, m, b : b + 1],
                            scale=1.0,
                        )

            # att^T columns: att[p] = sum_a s[a, p] * w_full[a]
            for bi in range(group):
                s_bf = s_tiles[bi]
                for m in range(AK):
                    nc.tensor.matmul(
                        attT0[:, bi : bi + 1],
                        lhsT=s_bf[:, m, 0:128],
                        rhs=w_full_bf[:, m : m + 1],
                        start=(m == 0),
                        stop=(m == AK - 1),
                    )
                for m in range(AK):
                    nc.tensor.matmul(
                        attT1[:, bi : bi + 1],
                        lhsT=s_bf[:, m, 128:P],
                        rhs=w_full_bf[:, m : m + 1],
                        start=(m == 0),
                        stop=(m == AK - 1),
                    )

            # ---- group softmax (no max-subtraction: |att| <~ 2, exp is safe) ----
            expT0 = sm_pool.tile([128, group], F32, tag="expT0")
            nc.scalar.activation(expT0[:], attT0[:], mybir.ActivationFunctionType.Exp)
            expT1 = sm_pool.tile([68, group], F32, tag="expT1")
            nc.scalar.activation(expT1[:], attT1[:], mybir.ActivationFunctionType.Exp)

            psR0 = small_ps.tile([group, 128], F32, tag="small")
            nc.tensor.transpose(psR0[:], expT0[:], identity[:128, :128])
            psR1 = small_ps.tile([group, 68], F32, tag="small")
            nc.tensor.transpose(psR1[:], expT1[:], identity[:68, :68])

            exp_rows = sm_pool.tile([group, P], F32, tag="exp_rows")
            nc.scalar.copy(exp_rows[:, 0:128], psR0[:])
            nc.scalar.copy(exp_rows[:, 128:P], psR1[:])

            sums = sm_pool.tile([group, 1], F32, tag="sums")
            nc.vector.tensor_reduce(
                sums[:], exp_rows[:], axis=mybir.AxisListType.X, op=mybir.AluOpType.add
            )
            rcp = sm_pool.tile([group, 1], F32, tag="rcp")
            nc.vector.reciprocal(rcp[:], sums[:])
            alpha_rows = sm_pool.tile([group, P], F32, tag="alpha_rows")
            nc.vector.tensor_scalar_mul(alpha_rows[:], exp_rows[:], rcp[:, 0:1])
            nc.sync.dma_start(alpha_out[g * group : (g + 1) * group, :], alpha_rows[:])

            # alpha^T in bf16 (stationary operand for awe matmuls)
            psT0 = small_ps.tile([128, group], F32, tag="small")
            nc.tensor.transpose(psT0[:], alpha_rows[:, 0:128], identity[:group, :group])
            alphaT0 = alphaT_pool.tile([128, group], BF16, tag="alphaT0")
            nc.scalar.copy(alphaT0[:], psT0[:])
            psT1 = small_ps.tile([68, group], F32, tag="small")
            nc.tensor.transpose(psT1[:], alpha_rows[:, 128:P], identity[:group, :group])
            alphaT1 = alphaT_pool.tile([68, group], BF16, tag="alphaT1")
            nc.scalar.copy(alphaT1[:], psT1[:])

            # ---- awe: awe[b, e] = sum_p alpha[b, p] * enc[b, p, e] ----
            awe_strip = awe_sb_pool.tile([1, group * E], F32, tag="awe_strip")
            for bi in range(group):
                for sl in range(E // 512):
                    psA = awe_ps.tile([1, 512], F32, tag="awe")
                    nc.tensor.matmul(
                        psA[:],
                        lhsT=alphaT0[:, bi : bi + 1],
                        rhs=enc_tiles[bi][:, 0, sl * 512 : (sl + 1) * 512],
                        start=True,
                        stop=False,
                    )
                    nc.tensor.matmul(
                        psA[:],
                        lhsT=alphaT1[:, bi : bi + 1],
                        rhs=enc_tiles[bi][:68, 1, sl * 512 : (sl + 1) * 512],
                        start=False,
                        stop=True,
                    )
                    evict = nc.scalar.copy if sl % 2 == 0 else nc.vector.tensor_copy
                    evict(
                        awe_strip[0:1, bi * E + sl * 512 : bi * E + (sl + 1) * 512],
                        psA[:],
                    )
            nc.sync.dma_start(
                awe_out[g * group : (g + 1) * group, :], awe_strip[0:1, :]
            )

    nc.compile()
    return nc


_NC_CACHE = {}


def _get_nc(b_loc, group):
    key = (b_loc, group)
    if key not in _NC_CACHE:
        _NC_CACHE[key] = build_nc(b_loc, group)
    return _NC_CACHE[key]


def run_spmd(inputs, trace=False, **kwargs):
    """Run on 8 NeuronCores; returns (awe, alpha, BassKernelResults)."""
    from concourse import bass_utils

    enc = np.asarray(inputs["encoder_out"], dtype=np.float32)
    dec = np.asarray(inputs["decoder_hidden"], dtype=np.float32)
    shared = {
        "W_enc": np.asarray(inputs["W_enc"], dtype=np.float32),
        "b_enc": np.asarray(inputs["b_enc"], dtype=np.float32),
        "W_dec": np.asarray(inputs["W_dec"], dtype=np.float32),
        "b_dec": np.asarray(inputs["b_dec"], dtype=np.float32),
        "W_full": np.asarray(inputs["W_full"], dtype=np.float32),
    }
    b_total = enc.shape[0]
    b_loc = b_total // N_CORES
    nc = _get_nc(b_loc, 4)

    in_maps = []
    for c in range(N_CORES):
        sl = slice(c * b_loc, (c + 1) * b_loc)
        m = dict(shared)
        m["encoder_out"] = np.ascontiguousarray(enc[sl])
        m["decoder_hidden"] = np.ascontiguousarray(dec[sl])
        in_maps.append(m)

    res = bass_utils.run_bass_kernel_spmd(
        nc, in_maps, list(range(N_CORES)), trace=trace, **kwargs
    )
    awe = np.concatenate([r["awe"] for r in res.results], axis=0)
    alpha = np.concatenate([r["alpha"] for r in res.results], axis=0)
    return awe, alpha, res


def kernel(**inputs):
    awe, alpha, _ = run_spmd(inputs)
    return awe, alpha
